# revision 36
# baseline (speedup 1.0000x reference)
"""Channel-attention kernel for Trainium2, SPMD across 8 NeuronCores.

Problem: x:[4,512,64,64] f32; q = wq@x+bq, k = wk@x+bk (Cq=64), v = wv@x+bv;
scores = q^T k -> [B,4096,4096]; attn = softmax(scores, -1);
out = v @ attn^T; y = gamma*out + x.

Sharding: 8 shards = 4 batches x 2 query-halves. Each core gets its batch's
x pre-rotated along the pixel axis so its 2048 queries sit in columns 0:2048
(softmax/AV are permutation-invariant over keys, so rotating keys/values is
harmless). This keeps the SPMD program identical on every core.

Host marshalling (np.roll of x per core was already required by the
sharding, so the host also pre-packs the compute images): x ships twice --
once as fp8e4 in the DoubleRow-pair layout (2MB, feeds QK/V projections)
and once as bf16 (4MB, feeds only the late residual epilogue) -- and the
tiny weights ship fp8 DR-packed / transposed.  That removes every cast and
transpose from the device's critical path: the exp stream is gated only by
the small fp8 image.

Per-core pipeline (v12):
  P1: xp-fp8 streams on sync+gpsimd; per 512-px slab: QK projection in fp8
      DoubleRow (2 matmuls), bias-add on ACT, k2lo/q2hi partition-split
      copies, V-projection (lagging two slabs, PSUM drains on DVE), and
      group-0 score pairs + exp + a j-by-j denominator chain.  The bf16
      residual x trickles in behind the fp8 traffic.
  P2: three group slots.  Slot g streams scores(g+1)+exp(g+1) interleaved
      in jj-pairs with AV(g) half-chains (8 fp8-DR matmuls each, dense
      enough to keep the PE's HAM clock warm), the g+1 denominator chain
      rides two pairs behind the exp stream, the reciprocal runs hidden
      under the first AV chunk (reciprocal_approx_fast), and each ct's
      epilogue (av*gamma/d + gamma*bv + x) trails its chain.
  Tail: AV(3) + epilogue only.
"""

import numpy as np
import ml_dtypes

import concourse.bass as bass
import concourse.bacc as bacc
import concourse.mybir as mybir
import concourse.tile as tile
from concourse import bass_utils, masks

B, C, W, H = 4, 512, 64, 64
N = W * H          # 4096 pixels
CQ = 64            # query/key channels
NH = N // 2        # 2048 queries per core
NCORES = 8
F32 = mybir.dt.float32
BF16 = mybir.dt.bfloat16
FP8E4 = mybir.dt.float8e4
FP8E5 = mybir.dt.float8e5
DR = mybir.MatmulPerfMode.DoubleRow
VPAD = 528   # fp8 vT pair stride, %16 == 0
AF = mybir.ActivationFunctionType
MUL = mybir.AluOpType.mult
ADD = mybir.AluOpType.add

NJ = 16            # key-tile pairs
N_G = NH // 512    # 4 query groups per core
NS = 8             # x column slabs of 512 pixels


def _emit(tc, x, xp0, xp1, wqkT8d, wvT8d, bqk, bvT, gamma, y):
    nc = tc.nc

    with (
        tc.tile_pool(name="const", bufs=1) as const,
        tc.tile_pool(name="data", bufs=1) as data,
    ):
        # ---- constants (gpsimd memsets, no deps) -------------------------
        ones_f32 = const.tile([1, 128], F32, tag="ones")
        nc.gpsimd.memset(ones_f32[:], 1.0)
        nbias = const.tile([128, 1], F32, tag="nbias")
        nc.gpsimd.memset(nbias[:], -4.0)
        onesP = const.tile([128, 32], FP8E4, tag="onesP")
        nc.gpsimd.memset(onesP[:], 1.0)

        # ---- persistent data ---------------------------------------------
        xf = [data.tile([128, N], BF16, tag=f"xf{r}", name=f"xf{r}")
              for r in range(4)]
        xp = [data.tile([128, 2 * N], FP8E4, tag=f"xp{pc}", name=f"xp{pc}")
              for pc in range(2)]
        qkb = data.tile([128, N], BF16, tag="qkb")
        k2lo = data.tile([64, N], BF16, tag="k2lo")
        q2hi = data.tile([128, NH], BF16, tag="q2hi")
        vP = [data.tile([128, 2 * VPAD], FP8E4, tag=f"vP{j}", name=f"vP{j}")
              for j in range(NJ)]
        wqkT8 = data.tile([128, 512], FP8E4, tag="wqkT8")
        wvTp = data.tile([128, 2048], FP8E4, tag="wvTp")
        gones = const.tile([1, 128], F32, tag="gones")
        gammab = const.tile([128, 1], F32, tag="gammab")
        gbv = const.tile([128, 4], F32, tag="gbv")
        bqk_s = const.tile([128, 1], F32, tag="bqk")
        bvT_s = const.tile([128, 4], F32, tag="bvT")
        g_s = const.tile([1, 1], F32, tag="gs")

        # ---- DMA plan ----------------------------------------------------
        # fp8 x image: slab-0 pieces first, then two 1792-px tail chunks
        # per (pc, half); sync carries xp0, gpsimd carries xp1.
        xpd = (xp0, xp1)

        def xp_piece(eng, pc, i, lo, w):
            eng.dma_start(xp[pc][:, i * N + lo:i * N + lo + w],
                          xpd[pc][:, i * N + lo:i * N + lo + w])

        for i in range(2):
            xp_piece(nc.sync, 0, i, 0, 512)
            xp_piece(nc.gpsimd, 1, i, 0, 512)
        for c in range(2):
            xp_piece(nc.sync, 0, 0, 512 + c * 1792, 1792)
            xp_piece(nc.gpsimd, 1, 0, 512 + c * 1792, 1792)
        # weights on scalar: all tiny, hw queue drains in a few us
        nc.scalar.dma_start(wqkT8[:], wqkT8d)
        nc.scalar.dma_start(bqk_s[:], bqk)
        nc.scalar.dma_start(bvT_s[:], bvT)
        nc.scalar.dma_start(g_s[:], gamma)
        nc.scalar.dma_start(wvTp[:], wvT8d)
        # the second DR halves of the xp tails ride 3-way across queues
        for c in range(2):
            xp_piece(nc.scalar, 0, 1, 512 + c * 1792, 1792)
            xp_piece(nc.sync, 1, 1, 512 + c * 1792, 1792) if False else None
            xp_piece(nc.gpsimd, 1, 1, 512 + c * 1792, 1792)
        # bf16 residual x: not needed until the first epilogue; it rides
        # behind the fp8 traffic in whole-block transfers
        nc.gpsimd.dma_start(xf[2][:], x[256:384, :])
        nc.gpsimd.dma_start(xf[3][:], x[384:512, :])

        def alloc_expP(g):
            return [data.tile([128, 1024], FP8E5, tag=f"expP{j}",
                              name=f"expP{j}_{g}", bufs=2)
                    for j in range(NJ)]

        with (
            tc.tile_pool(name="psSC", bufs=2, space="PSUM") as psSC,
            tc.tile_pool(name="psD", bufs=1, space="PSUM") as psD,
        ):
            ones_ap = onesP[:].rearrange("p (i n) -> p i n", i=2)[:, :, 0:1]

            def score_pair(expP_list, g, j):
                mA, mB = 2 * j, 2 * j + 1
                ps = psSC.tile([128, 1024], F32, tag="sc",
                               name=f"ps{g}_{j}")
                nc.tensor.matmul(
                    ps[:, 0:512], k2lo[:, mA * 128:(mA + 1) * 128],
                    qkb[0:CQ, g * 512:(g + 1) * 512],
                    start=True, stop=True,
                )
                nc.tensor.matmul(
                    ps[:, 512:1024],
                    qkb[CQ:128, mB * 128:(mB + 1) * 128],
                    q2hi[CQ:128, g * 512:(g + 1) * 512],
                    start=True, stop=True,
                )
                nc.scalar.activation(expP_list[j][:], ps[:], AF.Exp,
                                     bias=nbias[:])

            def dn_link(dt, expP_list, j):
                nc.tensor.matmul(
                    dt[0:1, :], ones_ap,
                    expP_list[j][:].rearrange("p (i n) -> p i n", i=2),
                    start=(j == 0), stop=(j == NJ - 1), perf_mode=DR,
                )

            # ================= P1: slab-streamed prologue =================
            with (
                tc.tile_pool(name="psQK", bufs=1, space="PSUM") as psQK,
                tc.tile_pool(name="psV", bufs=2, space="PSUM") as psV,
            ):
                expP = alloc_expP(0)

                def v_pair(j):
                    for half in range(2):
                        mt = 2 * j + half
                        ps = psV.tile([128, 512], F32, tag="v",
                                      name=f"vps{j}_{half}")
                        for pc in range(2):
                            lhx = xp[pc][:].rearrange(
                                "p (i n) -> p i n", i=2)[
                                :, :, mt * 128:(mt + 1) * 128]
                            wvr = wvTp[:].rearrange(
                                "p (c i n) -> p c i n", c=2, i=2)[:, pc]
                            nc.tensor.matmul(
                                ps[:], lhx, wvr,
                                start=(pc == 0), stop=(pc == 1),
                                perf_mode=DR,
                            )
                        nc.vector.tensor_copy(
                            vP[j][:, half * VPAD:half * VPAD + 512], ps[:])

                def slab_front(s):
                    """fp8 DR QK + bias on ACT + partition-split copies"""
                    lo = s * 512
                    qps = psQK.tile([128, 512], F32, tag="qk",
                                    name=f"qps{s}")
                    for pc in range(2):
                        mv = xp[pc][:].rearrange(
                            "p (i n) -> p i n", i=2)[:, :, lo:lo + 512]
                        st = wqkT8[:].rearrange(
                            "p (c i n) -> p c i n", c=2, i=2)[:, pc]
                        nc.tensor.matmul(qps[:], st, mv,
                                         start=(pc == 0), stop=(pc == 1),
                                         perf_mode=DR)
                    nc.vector.tensor_scalar_add(
                        qkb[:, lo:lo + 512], qps[:], bqk_s[:])
                    ceng = nc.scalar
                    ceng.dma_start(
                        k2lo[:, lo:lo + 512], qkb[CQ:128, lo:lo + 512])
                    if s < 4:
                        ceng.dma_start(
                            q2hi[CQ:128, lo:lo + 512],
                            qkb[0:CQ, lo:lo + 512])

                # -- slabs 0,1 + epilogue constants --
                slab_front(0)
                score_pair(expP, 0, 0)
                score_pair(expP, 0, 1)
                slab_front(1)
                nc.vector.tensor_scalar_mul(gones[:], ones_f32[:], g_s[:])
                pg = psD.tile([128, 4], F32, tag="d", name="pg")
                nc.tensor.matmul(pg[:, 0:1], ones_f32[:], g_s[:],
                                 start=True, stop=True)
                nc.vector.tensor_copy(gammab[:], pg[:, 0:1])
                nc.vector.tensor_scalar_mul(gbv[:], bvT_s[:], gammab[:])
                score_pair(expP, 0, 2)
                score_pair(expP, 0, 3)

                # -- slabs 2..7: v-pairs and the g0 denom chain lag two --
                dt = psD.tile([128, 512], F32, tag="d", name="d0")
                for s in range(2, NS):
                    slab_front(s)
                    for j in (2 * s - 4, 2 * s - 3):
                        v_pair(j)
                    score_pair(expP, 0, 2 * s)
                    score_pair(expP, 0, 2 * s + 1)
                    dn_link(dt, expP, 2 * s - 4)
                    dn_link(dt, expP, 2 * s - 3)
                # bf16 residual x blocks 0,1 ride sync after all the
                # latency-critical fp8/copy traffic
                nc.sync.dma_start(xf[0][:], x[0:128, :])
                nc.sync.dma_start(xf[1][:], x[128:256, :])
                for j in (12, 13, 14, 15):
                    v_pair(j)
                    dn_link(dt, expP, j)

            # ============== P2: group slots + tail ========================
            with (
                tc.tile_pool(name="psAV", bufs=3, space="PSUM") as psAV,
                tc.tile_pool(name="small", bufs=2) as small,
                tc.tile_pool(name="yout", bufs=2) as yout,
            ):
                for g in range(N_G):
                    nxt = alloc_expP(g + 1) if g + 1 < N_G else None
                    dt_nxt = (psD.tile([128, 512], F32, tag="d",
                                       name=f"d{g + 1}")
                              if nxt is not None else None)
                    gcols = slice(g * 512, (g + 1) * 512)
                    shift = 2 if g == 0 else 0
                    dr = gdbs = av = None

                    def av_half(ct, half):
                        for j in range(half * 8, half * 8 + 8):
                            vst = vP[j][:].rearrange(
                                "p (i n) -> p i n", i=2)[
                                :, :, ct * 128:(ct + 1) * 128]
                            nc.tensor.matmul(
                                av[:], vst,
                                expP[j][:].rearrange("p (i n) -> p i n",
                                                     i=2),
                                start=(j == 0), stop=(j == NJ - 1),
                                perf_mode=DR,
                            )

                    def epilogue(ct):
                        tmp = yout.tile([128, 512], F32, tag="tmp")
                        nc.vector.tensor_tensor(tmp[:], av[:], gdbs[:], MUL)
                        yo = yout.tile([128, 512], F32, tag="yo")
                        # yo = (tmp + gamma*bv) + x   (x bf16 in SBUF)
                        nc.vector.scalar_tensor_tensor(
                            yo[:], tmp[:], gbv[:, ct:ct + 1],
                            xf[ct][:, gcols], ADD, ADD)
                        eng = nc.sync if ct % 2 == 0 else nc.gpsimd
                        eng.dma_start(
                            y[ct * 128:(ct + 1) * 128, gcols], yo[:])

                    for p in range(8):          # jj pairs
                        if nxt is not None:
                            score_pair(nxt, g + 1, 2 * p)
                            score_pair(nxt, g + 1, 2 * p + 1)
                            if p >= 1:
                                dn_link(dt_nxt, nxt, 2 * p - 2)
                                dn_link(dt_nxt, nxt, 2 * p - 1)
                        if p == 0:
                            dr = small.tile([1, 512], F32, tag="dr")
                            with nc.allow_low_precision(
                                    reason="approx 1/d; rescaled by gamma"):
                                nc.vector.reciprocal_approx_fast(
                                    dr[:], dt[0:1, :])
                        if p >= shift:
                            ct, half = (p - shift) // 2, (p - shift) % 2
                            if half == 0:
                                av = psAV.tile([128, 512], F32, tag="av",
                                               name=f"av{g}_{ct}")
                            av_half(ct, half)
                        if p == 1:
                            gdb = psAV.tile([128, 512], F32, tag="av",
                                            name=f"gdb{g}")
                            nc.tensor.matmul(gdb[:], gones[:], dr[:],
                                             start=True, stop=True)
                            gdbs = small.tile([128, 512], F32, tag="gdbs",
                                              bufs=2)
                            nc.vector.tensor_copy(gdbs[:], gdb[:])
                        if p >= shift and (p - shift) % 2 == 1:
                            epilogue((p - shift) // 2)
                    if shift:
                        av = psAV.tile([128, 512], F32, tag="av",
                                       name=f"av{g}_3")
                        av_half(3, 0)
                        av_half(3, 1)
                        epilogue(3)
                    if nxt is not None:
                        dn_link(dt_nxt, nxt, 14)
                        dn_link(dt_nxt, nxt, 15)
                    dt = dt_nxt
                    expP = nxt


def build_nc():
    nc = bacc.Bacc("TRN2", target_bir_lowering=False, debug=False,
                   num_devices=NCORES)
    x = nc.dram_tensor("x", [C, N], BF16, kind="ExternalInput")
    xp0 = nc.dram_tensor("xp0", [128, 2 * N], FP8E4, kind="ExternalInput")
    xp1 = nc.dram_tensor("xp1", [128, 2 * N], FP8E4, kind="ExternalInput")
    wqkT8d = nc.dram_tensor("wqkT8", [128, 512], FP8E4,
                            kind="ExternalInput")
    wvT8d = nc.dram_tensor("wvT8", [128, 2048], FP8E4,
                           kind="ExternalInput")
    bqk = nc.dram_tensor("bqk", [128, 1], F32, kind="ExternalInput")
    bvT = nc.dram_tensor("bvT", [128, 4], F32, kind="ExternalInput")
    gamma = nc.dram_tensor("gamma", [1, 1], F32, kind="ExternalInput")
    y = nc.dram_tensor("y", [C, NH], F32, kind="ExternalOutput")
    with tile.TileContext(nc) as tc:
        _emit(tc, x.ap(), xp0.ap(), xp1.ap(), wqkT8d.ap(), wvT8d.ap(),
              bqk.ap(), bvT.ap(), gamma.ap(), y.ap())
    nc.compile()
    return nc


def make_in_maps(inputs):
    FP8 = ml_dtypes.float8_e4m3
    BF = ml_dtypes.bfloat16
    xfull = np.ascontiguousarray(
        np.asarray(inputs["x"], dtype=np.float32).reshape(B, C, N))
    wq = np.asarray(inputs["wq"], dtype=np.float32)
    wk = np.asarray(inputs["wk"], dtype=np.float32)
    wqkT = np.concatenate([wq, wk], axis=0).T      # [C, 128]
    wqkT8 = np.ascontiguousarray(np.concatenate(
        [wqkT[0:128], wqkT[128:256], wqkT[256:384], wqkT[384:512]],
        axis=1).astype(FP8))                       # [128, 512]
    wvT = np.asarray(inputs["wv"], dtype=np.float32).T   # [C, C]
    wvT8 = np.ascontiguousarray(np.concatenate(
        [wvT[0:128], wvT[128:256], wvT[256:384], wvT[384:512]],
        axis=1).astype(FP8))                       # [128, 2048]
    bqk = np.concatenate([
        np.asarray(inputs["bq"], dtype=np.float32),
        np.asarray(inputs["bk"], dtype=np.float32),
    ]).reshape(128, 1)
    bvT = np.ascontiguousarray(
        np.asarray(inputs["bv"], dtype=np.float32).reshape(4, 128).T)
    gamma = np.asarray(inputs["gamma"], dtype=np.float32).reshape(1, 1)
    in_maps = []
    for i in range(NCORES):
        b, h = divmod(i, 2)
        xr = np.roll(xfull[b], -h * NH, axis=1) if h else xfull[b]
        xp0 = np.ascontiguousarray(np.concatenate(
            [xr[0:128], xr[128:256]], axis=1).astype(FP8))
        xp1 = np.ascontiguousarray(np.concatenate(
            [xr[256:384], xr[384:512]], axis=1).astype(FP8))
        in_maps.append({
            "x": np.ascontiguousarray(xr.astype(BF)),
            "xp0": xp0, "xp1": xp1, "wqkT8": wqkT8, "wvT8": wvT8,
            "bqk": bqk, "bvT": bvT, "gamma": gamma,
        })
    return in_maps


_NC = None


def _get_nc():
    global _NC
    if _NC is None:
        _NC = build_nc()
    return _NC


def kernel(**inputs):
    nc = _get_nc()
    in_maps = make_in_maps(inputs)
    res = bass_utils.run_bass_kernel_spmd(nc, in_maps, core_ids=list(range(NCORES)))
    yf = np.empty((B, C, N), dtype=np.float32)
    for i in range(NCORES):
        b, h = divmod(i, 2)
        yf[b][:, h * NH:(h + 1) * NH] = res.results[i]["y"]
    return yf.reshape(B, C, W, H)


# revision 37
# speedup vs baseline: 1.0367x; 1.0367x over previous
"""Channel-attention kernel for Trainium2, SPMD across 8 NeuronCores.

Problem: x:[4,512,64,64] f32; q = wq@x+bq, k = wk@x+bk (Cq=64), v = wv@x+bv;
scores = q^T k -> [B,4096,4096]; attn = softmax(scores, -1);
out = v @ attn^T; y = gamma*out + x.

Sharding: 8 shards = 4 batches x 2 query-halves. Each core gets its batch's
x pre-rotated along the pixel axis so its 2048 queries sit in columns 0:2048
(softmax/AV are permutation-invariant over keys, so rotating keys/values is
harmless). This keeps the SPMD program identical on every core.

Host marshalling (np.roll of x per core was already required by the
sharding, so the host also pre-packs the compute images): x ships twice --
once as fp8e4 in the DoubleRow-pair layout (2MB, feeds QK/V projections)
and once as bf16 (4MB, feeds only the late residual epilogue) -- and the
tiny weights ship fp8 DR-packed / transposed.  That removes every cast and
transpose from the device's critical path: the exp stream is gated only by
the small fp8 image.

Per-core pipeline (v12):
  P1: xp-fp8 streams on sync+gpsimd; per 512-px slab: QK projection in fp8
      DoubleRow (2 matmuls), bias-add on ACT, k2lo/q2hi partition-split
      copies, V-projection (lagging two slabs, PSUM drains on DVE), and
      group-0 score pairs + exp + a j-by-j denominator chain.  The bf16
      residual x trickles in behind the fp8 traffic.
  P2: three group slots.  Slot g streams scores(g+1)+exp(g+1) interleaved
      in jj-pairs with AV(g) half-chains (8 fp8-DR matmuls each, dense
      enough to keep the PE's HAM clock warm), the g+1 denominator chain
      rides two pairs behind the exp stream, the reciprocal runs hidden
      under the first AV chunk (reciprocal_approx_fast), and each ct's
      epilogue (av*gamma/d + gamma*bv + x) trails its chain.
  Tail: AV(3) + epilogue only.
"""

import numpy as np
import ml_dtypes

import concourse.bass as bass
import concourse.bacc as bacc
import concourse.mybir as mybir
import concourse.tile as tile
from concourse import bass_utils, masks

B, C, W, H = 4, 512, 64, 64
N = W * H          # 4096 pixels
CQ = 64            # query/key channels
NH = N // 2        # 2048 queries per core
NCORES = 8
F32 = mybir.dt.float32
BF16 = mybir.dt.bfloat16
FP8E4 = mybir.dt.float8e4
FP8E5 = mybir.dt.float8e5
DR = mybir.MatmulPerfMode.DoubleRow
VPAD = 528   # fp8 vT pair stride, %16 == 0
AF = mybir.ActivationFunctionType
MUL = mybir.AluOpType.mult
ADD = mybir.AluOpType.add

NJ = 16            # key-tile pairs
N_G = NH // 512    # 4 query groups per core
NS = 8             # x column slabs of 512 pixels


def _emit(tc, x, xp0, xp1, wqkT8d, wvT8d, bqk, bvT, gamma, y):
    nc = tc.nc

    with (
        tc.tile_pool(name="const", bufs=1) as const,
        tc.tile_pool(name="data", bufs=1) as data,
    ):
        # ---- constants (gpsimd memsets, no deps) -------------------------
        ones_f32 = const.tile([1, 128], F32, tag="ones")
        nc.gpsimd.memset(ones_f32[:], 1.0)
        nbias = const.tile([128, 1], F32, tag="nbias")
        nc.gpsimd.memset(nbias[:], -4.0)
        onesP = const.tile([128, 32], FP8E4, tag="onesP")
        nc.gpsimd.memset(onesP[:], 1.0)

        # ---- persistent data ---------------------------------------------
        xf = [data.tile([128, N], BF16, tag=f"xf{r}", name=f"xf{r}")
              for r in range(4)]
        xp = [data.tile([128, 2 * N], FP8E4, tag=f"xp{pc}", name=f"xp{pc}")
              for pc in range(2)]
        qkb = data.tile([128, N], BF16, tag="qkb")
        k2lo = data.tile([64, N], BF16, tag="k2lo")
        q2hi = data.tile([128, NH], BF16, tag="q2hi")
        vP = [data.tile([128, 2 * VPAD], FP8E4, tag=f"vP{j}", name=f"vP{j}")
              for j in range(NJ)]
        wqkT8 = data.tile([128, 512], FP8E4, tag="wqkT8")
        wvTp = data.tile([128, 2048], FP8E4, tag="wvTp")
        gones = const.tile([1, 128], F32, tag="gones")
        gammab = const.tile([128, 1], F32, tag="gammab")
        gbv = const.tile([128, 4], F32, tag="gbv")
        bqk_s = const.tile([128, 1], F32, tag="bqk")
        bvT_s = const.tile([128, 4], F32, tag="bvT")
        g_s = const.tile([1, 1], F32, tag="gs")

        # ---- DMA plan ----------------------------------------------------
        # fp8 x image: slab-0 pieces first, then two 1792-px tail chunks
        # per (pc, half); sync carries xp0, gpsimd carries xp1.
        xpd = (xp0, xp1)

        def xp_piece(eng, pc, i, lo, w):
            eng.dma_start(xp[pc][:, i * N + lo:i * N + lo + w],
                          xpd[pc][:, i * N + lo:i * N + lo + w])

        for i in range(2):
            xp_piece(nc.sync, 0, i, 0, 512)
            xp_piece(nc.gpsimd, 1, i, 0, 512)
        for c in range(2):
            for i in range(2):
                xp_piece(nc.sync, 0, i, 512 + c * 1792, 1792)
                xp_piece(nc.gpsimd, 1, i, 512 + c * 1792, 1792)
        # weights on scalar: all tiny, hw queue drains in a few us
        nc.scalar.dma_start(wqkT8[:], wqkT8d)
        nc.scalar.dma_start(bqk_s[:], bqk)
        nc.scalar.dma_start(bvT_s[:], bvT)
        nc.scalar.dma_start(g_s[:], gamma)
        nc.scalar.dma_start(wvTp[:], wvT8d)
        # bf16 residual x: not needed until the first epilogue; it rides
        # behind the fp8 traffic in whole-block transfers
        nc.gpsimd.dma_start(xf[2][:], x[256:384, :])
        nc.gpsimd.dma_start(xf[3][:], x[384:512, :])

        def alloc_expP(g):
            return [data.tile([128, 1024], FP8E5, tag=f"expP{j}",
                              name=f"expP{j}_{g}", bufs=2)
                    for j in range(NJ)]

        with (
            tc.tile_pool(name="psSC", bufs=2, space="PSUM") as psSC,
            tc.tile_pool(name="psD", bufs=1, space="PSUM") as psD,
        ):
            ones_ap = onesP[:].rearrange("p (i n) -> p i n", i=2)[:, :, 0:1]

            def score_pair(expP_list, g, j):
                mA, mB = 2 * j, 2 * j + 1
                ps = psSC.tile([128, 1024], F32, tag="sc",
                               name=f"ps{g}_{j}")
                nc.tensor.matmul(
                    ps[:, 0:512], k2lo[:, mA * 128:(mA + 1) * 128],
                    qkb[0:CQ, g * 512:(g + 1) * 512],
                    start=True, stop=True,
                )
                nc.tensor.matmul(
                    ps[:, 512:1024],
                    qkb[CQ:128, mB * 128:(mB + 1) * 128],
                    q2hi[CQ:128, g * 512:(g + 1) * 512],
                    start=True, stop=True,
                )
                nc.scalar.activation(expP_list[j][:], ps[:], AF.Exp,
                                     bias=nbias[:])

            def dn_link(dt, expP_list, j):
                nc.tensor.matmul(
                    dt[0:1, :], ones_ap,
                    expP_list[j][:].rearrange("p (i n) -> p i n", i=2),
                    start=(j == 0), stop=(j == NJ - 1), perf_mode=DR,
                )

            # ================= P1: slab-streamed prologue =================
            with (
                tc.tile_pool(name="psQK", bufs=1, space="PSUM") as psQK,
                tc.tile_pool(name="psV", bufs=2, space="PSUM") as psV,
            ):
                expP = alloc_expP(0)

                def v_pair(j):
                    for half in range(2):
                        mt = 2 * j + half
                        ps = psV.tile([128, 512], F32, tag="v",
                                      name=f"vps{j}_{half}")
                        for pc in range(2):
                            lhx = xp[pc][:].rearrange(
                                "p (i n) -> p i n", i=2)[
                                :, :, mt * 128:(mt + 1) * 128]
                            wvr = wvTp[:].rearrange(
                                "p (c i n) -> p c i n", c=2, i=2)[:, pc]
                            nc.tensor.matmul(
                                ps[:], lhx, wvr,
                                start=(pc == 0), stop=(pc == 1),
                                perf_mode=DR,
                            )
                        nc.vector.tensor_copy(
                            vP[j][:, half * VPAD:half * VPAD + 512], ps[:])

                def slab_front(s):
                    """fp8 DR QK + bias on ACT + partition-split copies"""
                    lo = s * 512
                    qps = psQK.tile([128, 512], F32, tag="qk",
                                    name=f"qps{s}")
                    for pc in range(2):
                        mv = xp[pc][:].rearrange(
                            "p (i n) -> p i n", i=2)[:, :, lo:lo + 512]
                        st = wqkT8[:].rearrange(
                            "p (c i n) -> p c i n", c=2, i=2)[:, pc]
                        nc.tensor.matmul(qps[:], st, mv,
                                         start=(pc == 0), stop=(pc == 1),
                                         perf_mode=DR)
                    nc.vector.tensor_scalar_add(
                        qkb[:, lo:lo + 512], qps[:], bqk_s[:])
                    ceng = nc.scalar
                    ceng.dma_start(
                        k2lo[:, lo:lo + 512], qkb[CQ:128, lo:lo + 512])
                    if s < 4:
                        ceng.dma_start(
                            q2hi[CQ:128, lo:lo + 512],
                            qkb[0:CQ, lo:lo + 512])

                # -- slabs 0,1 + epilogue constants --
                slab_front(0)
                score_pair(expP, 0, 0)
                score_pair(expP, 0, 1)
                slab_front(1)
                nc.vector.tensor_scalar_mul(gones[:], ones_f32[:], g_s[:])
                pg = psD.tile([128, 4], F32, tag="d", name="pg")
                nc.tensor.matmul(pg[:, 0:1], ones_f32[:], g_s[:],
                                 start=True, stop=True)
                nc.vector.tensor_copy(gammab[:], pg[:, 0:1])
                nc.vector.tensor_scalar_mul(gbv[:], bvT_s[:], gammab[:])
                score_pair(expP, 0, 2)
                score_pair(expP, 0, 3)

                # -- slabs 2..7: v-pairs and the g0 denom chain lag two --
                dt = psD.tile([128, 512], F32, tag="d", name="d0")
                for s in range(2, NS):
                    slab_front(s)
                    for j in (2 * s - 4, 2 * s - 3):
                        v_pair(j)
                    score_pair(expP, 0, 2 * s)
                    score_pair(expP, 0, 2 * s + 1)
                    dn_link(dt, expP, 2 * s - 4)
                    dn_link(dt, expP, 2 * s - 3)
                # bf16 residual x blocks 0,1 ride sync after all the
                # latency-critical fp8/copy traffic
                nc.sync.dma_start(xf[0][:], x[0:128, :])
                nc.sync.dma_start(xf[1][:], x[128:256, :])
                for j in (12, 13, 14, 15):
                    v_pair(j)
                    dn_link(dt, expP, j)

            # ============== P2: group slots + tail ========================
            with (
                tc.tile_pool(name="psAV", bufs=3, space="PSUM") as psAV,
                tc.tile_pool(name="small", bufs=2) as small,
                tc.tile_pool(name="yout", bufs=2) as yout,
            ):
                for g in range(N_G):
                    nxt = alloc_expP(g + 1) if g + 1 < N_G else None
                    dt_nxt = (psD.tile([128, 512], F32, tag="d",
                                       name=f"d{g + 1}")
                              if nxt is not None else None)
                    gcols = slice(g * 512, (g + 1) * 512)
                    shift = 2 if g == 0 else 0
                    dr = gdbs = av = None

                    def av_half(ct, half):
                        for j in range(half * 8, half * 8 + 8):
                            vst = vP[j][:].rearrange(
                                "p (i n) -> p i n", i=2)[
                                :, :, ct * 128:(ct + 1) * 128]
                            nc.tensor.matmul(
                                av[:], vst,
                                expP[j][:].rearrange("p (i n) -> p i n",
                                                     i=2),
                                start=(j == 0), stop=(j == NJ - 1),
                                perf_mode=DR,
                            )

                    def epilogue(ct):
                        tmp = yout.tile([128, 512], F32, tag="tmp")
                        nc.vector.tensor_tensor(tmp[:], av[:], gdbs[:], MUL)
                        yo = yout.tile([128, 512], F32, tag="yo")
                        # yo = (tmp + gamma*bv) + x   (x bf16 in SBUF)
                        nc.vector.scalar_tensor_tensor(
                            yo[:], tmp[:], gbv[:, ct:ct + 1],
                            xf[ct][:, gcols], ADD, ADD)
                        eng = nc.sync if ct % 2 == 0 else nc.gpsimd
                        eng.dma_start(
                            y[ct * 128:(ct + 1) * 128, gcols], yo[:])

                    for p in range(8):          # jj pairs
                        if nxt is not None:
                            score_pair(nxt, g + 1, 2 * p)
                            score_pair(nxt, g + 1, 2 * p + 1)
                            if p >= 1:
                                dn_link(dt_nxt, nxt, 2 * p - 2)
                                dn_link(dt_nxt, nxt, 2 * p - 1)
                        if p == 0:
                            dr = small.tile([1, 512], F32, tag="dr")
                            with nc.allow_low_precision(
                                    reason="approx 1/d; rescaled by gamma"):
                                nc.vector.reciprocal_approx_fast(
                                    dr[:], dt[0:1, :])
                        if p >= shift:
                            ct, half = (p - shift) // 2, (p - shift) % 2
                            if half == 0:
                                av = psAV.tile([128, 512], F32, tag="av",
                                               name=f"av{g}_{ct}")
                            av_half(ct, half)
                        if p == 1:
                            gdb = psAV.tile([128, 512], F32, tag="av",
                                            name=f"gdb{g}")
                            nc.tensor.matmul(gdb[:], gones[:], dr[:],
                                             start=True, stop=True)
                            gdbs = small.tile([128, 512], F32, tag="gdbs",
                                              bufs=2)
                            nc.vector.tensor_copy(gdbs[:], gdb[:])
                        if p >= shift and (p - shift) % 2 == 1:
                            epilogue((p - shift) // 2)
                    if shift:
                        av = psAV.tile([128, 512], F32, tag="av",
                                       name=f"av{g}_3")
                        av_half(3, 0)
                        av_half(3, 1)
                        epilogue(3)
                    if nxt is not None:
                        dn_link(dt_nxt, nxt, 14)
                        dn_link(dt_nxt, nxt, 15)
                    dt = dt_nxt
                    expP = nxt


def build_nc():
    nc = bacc.Bacc("TRN2", target_bir_lowering=False, debug=False,
                   num_devices=NCORES)
    x = nc.dram_tensor("x", [C, N], BF16, kind="ExternalInput")
    xp0 = nc.dram_tensor("xp0", [128, 2 * N], FP8E4, kind="ExternalInput")
    xp1 = nc.dram_tensor("xp1", [128, 2 * N], FP8E4, kind="ExternalInput")
    wqkT8d = nc.dram_tensor("wqkT8", [128, 512], FP8E4,
                            kind="ExternalInput")
    wvT8d = nc.dram_tensor("wvT8", [128, 2048], FP8E4,
                           kind="ExternalInput")
    bqk = nc.dram_tensor("bqk", [128, 1], F32, kind="ExternalInput")
    bvT = nc.dram_tensor("bvT", [128, 4], F32, kind="ExternalInput")
    gamma = nc.dram_tensor("gamma", [1, 1], F32, kind="ExternalInput")
    y = nc.dram_tensor("y", [C, NH], F32, kind="ExternalOutput")
    with tile.TileContext(nc) as tc:
        _emit(tc, x.ap(), xp0.ap(), xp1.ap(), wqkT8d.ap(), wvT8d.ap(),
              bqk.ap(), bvT.ap(), gamma.ap(), y.ap())
    nc.compile()
    return nc


def make_in_maps(inputs):
    FP8 = ml_dtypes.float8_e4m3
    BF = ml_dtypes.bfloat16
    xfull = np.ascontiguousarray(
        np.asarray(inputs["x"], dtype=np.float32).reshape(B, C, N))
    wq = np.asarray(inputs["wq"], dtype=np.float32)
    wk = np.asarray(inputs["wk"], dtype=np.float32)
    wqkT = np.concatenate([wq, wk], axis=0).T      # [C, 128]
    wqkT8 = np.ascontiguousarray(np.concatenate(
        [wqkT[0:128], wqkT[128:256], wqkT[256:384], wqkT[384:512]],
        axis=1).astype(FP8))                       # [128, 512]
    wvT = np.asarray(inputs["wv"], dtype=np.float32).T   # [C, C]
    wvT8 = np.ascontiguousarray(np.concatenate(
        [wvT[0:128], wvT[128:256], wvT[256:384], wvT[384:512]],
        axis=1).astype(FP8))                       # [128, 2048]
    bqk = np.concatenate([
        np.asarray(inputs["bq"], dtype=np.float32),
        np.asarray(inputs["bk"], dtype=np.float32),
    ]).reshape(128, 1)
    bvT = np.ascontiguousarray(
        np.asarray(inputs["bv"], dtype=np.float32).reshape(4, 128).T)
    gamma = np.asarray(inputs["gamma"], dtype=np.float32).reshape(1, 1)
    in_maps = []
    for i in range(NCORES):
        b, h = divmod(i, 2)
        xr = np.roll(xfull[b], -h * NH, axis=1) if h else xfull[b]
        xp0 = np.ascontiguousarray(np.concatenate(
            [xr[0:128], xr[128:256]], axis=1).astype(FP8))
        xp1 = np.ascontiguousarray(np.concatenate(
            [xr[256:384], xr[384:512]], axis=1).astype(FP8))
        in_maps.append({
            "x": np.ascontiguousarray(xr.astype(BF)),
            "xp0": xp0, "xp1": xp1, "wqkT8": wqkT8, "wvT8": wvT8,
            "bqk": bqk, "bvT": bvT, "gamma": gamma,
        })
    return in_maps


_NC = None


def _get_nc():
    global _NC
    if _NC is None:
        _NC = build_nc()
    return _NC


def kernel(**inputs):
    nc = _get_nc()
    in_maps = make_in_maps(inputs)
    res = bass_utils.run_bass_kernel_spmd(nc, in_maps, core_ids=list(range(NCORES)))
    yf = np.empty((B, C, N), dtype=np.float32)
    for i in range(NCORES):
        b, h = divmod(i, 2)
        yf[b][:, h * NH:(h + 1) * NH] = res.results[i]["y"]
    return yf.reshape(B, C, W, H)


# revision 38
# speedup vs baseline: 1.0395x; 1.0027x over previous
"""Channel-attention kernel for Trainium2, SPMD across 8 NeuronCores.

Problem: x:[4,512,64,64] f32; q = wq@x+bq, k = wk@x+bk (Cq=64), v = wv@x+bv;
scores = q^T k -> [B,4096,4096]; attn = softmax(scores, -1);
out = v @ attn^T; y = gamma*out + x.

Sharding: 8 shards = 4 batches x 2 query-halves. Each core gets its batch's
x pre-rotated along the pixel axis so its 2048 queries sit in columns 0:2048
(softmax/AV are permutation-invariant over keys, so rotating keys/values is
harmless). This keeps the SPMD program identical on every core.

Host marshalling (np.roll of x per core was already required by the
sharding, so the host also pre-packs the compute images): x ships twice --
once as fp8e4 in the DoubleRow-pair layout (2MB, feeds QK/V projections)
and once as bf16 (4MB, feeds only the late residual epilogue) -- and the
tiny weights ship fp8 DR-packed / transposed.  That removes every cast and
transpose from the device's critical path: the exp stream is gated only by
the small fp8 image.

Per-core pipeline (v12):
  P1: xp-fp8 streams on sync+gpsimd; per 512-px slab: QK projection in fp8
      DoubleRow (2 matmuls), bias-add on ACT, k2lo/q2hi partition-split
      copies, V-projection (lagging two slabs, PSUM drains on DVE), and
      group-0 score pairs + exp + a j-by-j denominator chain.  The bf16
      residual x trickles in behind the fp8 traffic.
  P2: three group slots.  Slot g streams scores(g+1)+exp(g+1) interleaved
      in jj-pairs with AV(g) half-chains (8 fp8-DR matmuls each, dense
      enough to keep the PE's HAM clock warm), the g+1 denominator chain
      rides two pairs behind the exp stream, the reciprocal runs hidden
      under the first AV chunk (reciprocal_approx_fast), and each ct's
      epilogue (av*gamma/d + gamma*bv + x) trails its chain.
  Tail: AV(3) + epilogue only.
"""

import numpy as np
import ml_dtypes

import concourse.bass as bass
import concourse.bacc as bacc
import concourse.mybir as mybir
import concourse.tile as tile
from concourse import bass_utils, masks

B, C, W, H = 4, 512, 64, 64
N = W * H          # 4096 pixels
CQ = 64            # query/key channels
NH = N // 2        # 2048 queries per core
NCORES = 8
F32 = mybir.dt.float32
BF16 = mybir.dt.bfloat16
FP8E4 = mybir.dt.float8e4
FP8E5 = mybir.dt.float8e5
DR = mybir.MatmulPerfMode.DoubleRow
VPAD = 528   # fp8 vT pair stride, %16 == 0
AF = mybir.ActivationFunctionType
MUL = mybir.AluOpType.mult
ADD = mybir.AluOpType.add

NJ = 16            # key-tile pairs
N_G = NH // 512    # 4 query groups per core
NS = 8             # x column slabs of 512 pixels


def _emit(tc, x, xp0, xp1, wqkT8d, wvT8d, bqk, bvT, gamma, y):
    nc = tc.nc

    with (
        tc.tile_pool(name="const", bufs=1) as const,
        tc.tile_pool(name="data", bufs=1) as data,
    ):
        # ---- constants (gpsimd memsets, no deps) -------------------------
        ones_f32 = const.tile([1, 128], F32, tag="ones")
        nc.gpsimd.memset(ones_f32[:], 1.0)
        nbias = const.tile([128, 1], F32, tag="nbias")
        nc.gpsimd.memset(nbias[:], -4.0)
        onesP = const.tile([128, 32], FP8E4, tag="onesP")
        nc.gpsimd.memset(onesP[:], 1.0)
        ones_bf = const.tile([1, 128], F32, tag="onesbf")

        # ---- persistent data ---------------------------------------------
        xf = [data.tile([128, N], BF16, tag=f"xf{r}", name=f"xf{r}")
              for r in range(4)]
        xp = [data.tile([128, 2 * N], FP8E4, tag=f"xp{pc}", name=f"xp{pc}")
              for pc in range(2)]
        qkb = data.tile([128, N], BF16, tag="qkb")
        k2lo = data.tile([64, N], BF16, tag="k2lo")
        q2hi = data.tile([128, NH], BF16, tag="q2hi")
        vP = [data.tile([128, 2 * VPAD], FP8E4, tag=f"vP{j}", name=f"vP{j}")
              for j in range(NJ)]
        wqkT8 = data.tile([128, 512], FP8E4, tag="wqkT8")
        wvTp = data.tile([128, 2048], FP8E4, tag="wvTp")
        gones = const.tile([1, 128], BF16, tag="gones")
        gammab = const.tile([128, 1], F32, tag="gammab")
        gbv = const.tile([128, 4], F32, tag="gbv")
        bqk_s = const.tile([128, 1], F32, tag="bqk")
        bvT_s = const.tile([128, 4], F32, tag="bvT")
        g_s = const.tile([1, 1], F32, tag="gs")

        # ---- DMA plan ----------------------------------------------------
        # fp8 x image: slab-0 pieces first, then two 1792-px tail chunks
        # per (pc, half); sync carries xp0, gpsimd carries xp1.
        xpd = (xp0, xp1)

        def xp_piece(eng, pc, i, lo, w):
            eng.dma_start(xp[pc][:, i * N + lo:i * N + lo + w],
                          xpd[pc][:, i * N + lo:i * N + lo + w])

        for i in range(2):
            xp_piece(nc.sync, 0, i, 0, 512)
            xp_piece(nc.gpsimd, 1, i, 0, 512)
        for c in range(2):
            for i in range(2):
                xp_piece(nc.sync, 0, i, 512 + c * 1792, 1792)
                xp_piece(nc.gpsimd, 1, i, 512 + c * 1792, 1792)
        # weights on scalar: all tiny, hw queue drains in a few us
        nc.scalar.dma_start(wqkT8[:], wqkT8d)
        nc.scalar.dma_start(bqk_s[:], bqk)
        nc.scalar.dma_start(bvT_s[:], bvT)
        nc.scalar.dma_start(g_s[:], gamma)
        nc.scalar.dma_start(wvTp[:], wvT8d)
        # bf16 residual x: not needed until the first epilogue; it rides
        # behind the fp8 traffic in whole-block transfers
        nc.gpsimd.dma_start(xf[2][:], x[256:384, :])
        nc.gpsimd.dma_start(xf[3][:], x[384:512, :])

        def alloc_expP(g):
            return [data.tile([128, 1024], FP8E5, tag=f"expP{j}",
                              name=f"expP{j}_{g}", bufs=2)
                    for j in range(NJ)]

        with (
            tc.tile_pool(name="psSC", bufs=2, space="PSUM") as psSC,
            tc.tile_pool(name="psD", bufs=1, space="PSUM") as psD,
        ):
            ones_ap = onesP[:].rearrange("p (i n) -> p i n", i=2)[:, :, 0:1]

            def score_pair(expP_list, g, j):
                mA, mB = 2 * j, 2 * j + 1
                ps = psSC.tile([128, 1024], F32, tag="sc",
                               name=f"ps{g}_{j}")
                nc.tensor.matmul(
                    ps[:, 0:512], k2lo[:, mA * 128:(mA + 1) * 128],
                    qkb[0:CQ, g * 512:(g + 1) * 512],
                    start=True, stop=True,
                )
                nc.tensor.matmul(
                    ps[:, 512:1024],
                    qkb[CQ:128, mB * 128:(mB + 1) * 128],
                    q2hi[CQ:128, g * 512:(g + 1) * 512],
                    start=True, stop=True,
                )
                nc.scalar.activation(expP_list[j][:], ps[:], AF.Exp,
                                     bias=nbias[:])

            def dn_link(dt, expP_list, j):
                nc.tensor.matmul(
                    dt[0:1, :], ones_ap,
                    expP_list[j][:].rearrange("p (i n) -> p i n", i=2),
                    start=(j == 0), stop=(j == NJ - 1), perf_mode=DR,
                )

            # ================= P1: slab-streamed prologue =================
            with (
                tc.tile_pool(name="psQK", bufs=1, space="PSUM") as psQK,
                tc.tile_pool(name="psV", bufs=2, space="PSUM") as psV,
            ):
                expP = alloc_expP(0)

                def v_pair(j):
                    for half in range(2):
                        mt = 2 * j + half
                        ps = psV.tile([128, 512], F32, tag="v",
                                      name=f"vps{j}_{half}")
                        for pc in range(2):
                            lhx = xp[pc][:].rearrange(
                                "p (i n) -> p i n", i=2)[
                                :, :, mt * 128:(mt + 1) * 128]
                            wvr = wvTp[:].rearrange(
                                "p (c i n) -> p c i n", c=2, i=2)[:, pc]
                            nc.tensor.matmul(
                                ps[:], lhx, wvr,
                                start=(pc == 0), stop=(pc == 1),
                                perf_mode=DR,
                            )
                        nc.vector.tensor_copy(
                            vP[j][:, half * VPAD:half * VPAD + 512], ps[:])

                def slab_front(s):
                    """fp8 DR QK + bias on ACT + partition-split copies"""
                    lo = s * 512
                    qps = psQK.tile([128, 512], F32, tag="qk",
                                    name=f"qps{s}")
                    for pc in range(2):
                        mv = xp[pc][:].rearrange(
                            "p (i n) -> p i n", i=2)[:, :, lo:lo + 512]
                        st = wqkT8[:].rearrange(
                            "p (c i n) -> p c i n", c=2, i=2)[:, pc]
                        nc.tensor.matmul(qps[:], st, mv,
                                         start=(pc == 0), stop=(pc == 1),
                                         perf_mode=DR)
                    nc.vector.tensor_scalar_add(
                        qkb[:, lo:lo + 512], qps[:], bqk_s[:])
                    ceng = nc.scalar
                    ceng.dma_start(
                        k2lo[:, lo:lo + 512], qkb[CQ:128, lo:lo + 512])
                    if s < 4:
                        ceng.dma_start(
                            q2hi[CQ:128, lo:lo + 512],
                            qkb[0:CQ, lo:lo + 512])

                # -- slabs 0,1 + epilogue constants --
                slab_front(0)
                score_pair(expP, 0, 0)
                score_pair(expP, 0, 1)
                slab_front(1)
                nc.vector.tensor_scalar_mul(gones[:], ones_f32[:], g_s[:])
                pg = psD.tile([128, 4], F32, tag="d", name="pg")
                nc.tensor.matmul(pg[:, 0:1], ones_f32[:], g_s[:],
                                 start=True, stop=True)
                nc.vector.tensor_copy(gammab[:], pg[:, 0:1])
                nc.vector.tensor_scalar_mul(gbv[:], bvT_s[:], gammab[:])
                score_pair(expP, 0, 2)
                score_pair(expP, 0, 3)

                # -- slabs 2..7: v-pairs and the g0 denom chain lag two --
                dt = psD.tile([128, 512], F32, tag="d", name="d0")
                for s in range(2, NS):
                    slab_front(s)
                    for j in (2 * s - 4, 2 * s - 3):
                        v_pair(j)
                    score_pair(expP, 0, 2 * s)
                    score_pair(expP, 0, 2 * s + 1)
                    dn_link(dt, expP, 2 * s - 4)
                    dn_link(dt, expP, 2 * s - 3)
                # bf16 residual x blocks 0,1 ride sync after all the
                # latency-critical fp8/copy traffic
                nc.sync.dma_start(xf[0][:], x[0:128, :])
                nc.sync.dma_start(xf[1][:], x[128:256, :])
                for j in (12, 13, 14, 15):
                    v_pair(j)
                    dn_link(dt, expP, j)

            # ============== P2: group slots + tail ========================
            with (
                tc.tile_pool(name="psAV", bufs=3, space="PSUM") as psAV,
                tc.tile_pool(name="small", bufs=2) as small,
                tc.tile_pool(name="yout", bufs=2) as yout,
            ):
                for g in range(N_G):
                    nxt = alloc_expP(g + 1) if g + 1 < N_G else None
                    dt_nxt = (psD.tile([128, 512], F32, tag="d",
                                       name=f"d{g + 1}")
                              if nxt is not None else None)
                    gcols = slice(g * 512, (g + 1) * 512)
                    shift = 2 if g == 0 else 0
                    dr = gdbs = av = None

                    def av_half(ct, half):
                        for j in range(half * 8, half * 8 + 8):
                            vst = vP[j][:].rearrange(
                                "p (i n) -> p i n", i=2)[
                                :, :, ct * 128:(ct + 1) * 128]
                            nc.tensor.matmul(
                                av[:], vst,
                                expP[j][:].rearrange("p (i n) -> p i n",
                                                     i=2),
                                start=(j == 0), stop=(j == NJ - 1),
                                perf_mode=DR,
                            )

                    def epilogue(ct):
                        tmp = yout.tile([128, 512], F32, tag="tmp")
                        nc.vector.tensor_tensor(tmp[:], av[:], gdbs[:], MUL)
                        yo = yout.tile([128, 512], F32, tag="yo")
                        # yo = (tmp + gamma*bv) + x   (x bf16 in SBUF)
                        nc.vector.scalar_tensor_tensor(
                            yo[:], tmp[:], gbv[:, ct:ct + 1],
                            xf[ct][:, gcols], ADD, ADD)
                        eng = nc.sync if ct % 2 == 0 else nc.gpsimd
                        eng.dma_start(
                            y[ct * 128:(ct + 1) * 128, gcols], yo[:])

                    for p in range(8):          # jj pairs
                        if nxt is not None:
                            score_pair(nxt, g + 1, 2 * p)
                            score_pair(nxt, g + 1, 2 * p + 1)
                            if p >= 1:
                                dn_link(dt_nxt, nxt, 2 * p - 2)
                                dn_link(dt_nxt, nxt, 2 * p - 1)
                        if p == 0:
                            drf = small.tile([1, 512], F32, tag="drf")
                            with nc.allow_low_precision(
                                    reason="approx 1/d; rescaled by gamma"):
                                nc.vector.reciprocal_approx_fast(
                                    drf[:], dt[0:1, :])
                            dr = small.tile([1, 512], BF16, tag="dr")
                            with nc.allow_low_precision(
                                    reason="1/d to bf16 for a fast gdb MM"):
                                nc.vector.tensor_copy(dr[:], drf[:])
                        if p >= shift:
                            ct, half = (p - shift) // 2, (p - shift) % 2
                            if half == 0:
                                av = psAV.tile([128, 512], F32, tag="av",
                                               name=f"av{g}_{ct}")
                            av_half(ct, half)
                        if p == 1:
                            gdb = psAV.tile([128, 512], F32, tag="av",
                                            name=f"gdb{g}")
                            nc.tensor.matmul(gdb[:], gones[:], dr[:],
                                             start=True, stop=True)
                            gdbs = small.tile([128, 512], F32, tag="gdbs",
                                              bufs=2)
                            nc.vector.tensor_copy(gdbs[:], gdb[:])
                        if p >= shift and (p - shift) % 2 == 1:
                            epilogue((p - shift) // 2)
                    if shift:
                        av = psAV.tile([128, 512], F32, tag="av",
                                       name=f"av{g}_3")
                        av_half(3, 0)
                        av_half(3, 1)
                        epilogue(3)
                    if nxt is not None:
                        dn_link(dt_nxt, nxt, 14)
                        dn_link(dt_nxt, nxt, 15)
                    dt = dt_nxt
                    expP = nxt


def build_nc():
    nc = bacc.Bacc("TRN2", target_bir_lowering=False, debug=False,
                   num_devices=NCORES)
    x = nc.dram_tensor("x", [C, N], BF16, kind="ExternalInput")
    xp0 = nc.dram_tensor("xp0", [128, 2 * N], FP8E4, kind="ExternalInput")
    xp1 = nc.dram_tensor("xp1", [128, 2 * N], FP8E4, kind="ExternalInput")
    wqkT8d = nc.dram_tensor("wqkT8", [128, 512], FP8E4,
                            kind="ExternalInput")
    wvT8d = nc.dram_tensor("wvT8", [128, 2048], FP8E4,
                           kind="ExternalInput")
    bqk = nc.dram_tensor("bqk", [128, 1], F32, kind="ExternalInput")
    bvT = nc.dram_tensor("bvT", [128, 4], F32, kind="ExternalInput")
    gamma = nc.dram_tensor("gamma", [1, 1], F32, kind="ExternalInput")
    y = nc.dram_tensor("y", [C, NH], F32, kind="ExternalOutput")
    with tile.TileContext(nc) as tc:
        _emit(tc, x.ap(), xp0.ap(), xp1.ap(), wqkT8d.ap(), wvT8d.ap(),
              bqk.ap(), bvT.ap(), gamma.ap(), y.ap())
    nc.compile()
    return nc


def make_in_maps(inputs):
    FP8 = ml_dtypes.float8_e4m3
    BF = ml_dtypes.bfloat16
    xfull = np.ascontiguousarray(
        np.asarray(inputs["x"], dtype=np.float32).reshape(B, C, N))
    wq = np.asarray(inputs["wq"], dtype=np.float32)
    wk = np.asarray(inputs["wk"], dtype=np.float32)
    wqkT = np.concatenate([wq, wk], axis=0).T      # [C, 128]
    wqkT8 = np.ascontiguousarray(np.concatenate(
        [wqkT[0:128], wqkT[128:256], wqkT[256:384], wqkT[384:512]],
        axis=1).astype(FP8))                       # [128, 512]
    wvT = np.asarray(inputs["wv"], dtype=np.float32).T   # [C, C]
    wvT8 = np.ascontiguousarray(np.concatenate(
        [wvT[0:128], wvT[128:256], wvT[256:384], wvT[384:512]],
        axis=1).astype(FP8))                       # [128, 2048]
    bqk = np.concatenate([
        np.asarray(inputs["bq"], dtype=np.float32),
        np.asarray(inputs["bk"], dtype=np.float32),
    ]).reshape(128, 1)
    bvT = np.ascontiguousarray(
        np.asarray(inputs["bv"], dtype=np.float32).reshape(4, 128).T)
    gamma = np.asarray(inputs["gamma"], dtype=np.float32).reshape(1, 1)
    in_maps = []
    for i in range(NCORES):
        b, h = divmod(i, 2)
        xr = np.roll(xfull[b], -h * NH, axis=1) if h else xfull[b]
        xp0 = np.ascontiguousarray(np.concatenate(
            [xr[0:128], xr[128:256]], axis=1).astype(FP8))
        xp1 = np.ascontiguousarray(np.concatenate(
            [xr[256:384], xr[384:512]], axis=1).astype(FP8))
        in_maps.append({
            "x": np.ascontiguousarray(xr.astype(BF)),
            "xp0": xp0, "xp1": xp1, "wqkT8": wqkT8, "wvT8": wvT8,
            "bqk": bqk, "bvT": bvT, "gamma": gamma,
        })
    return in_maps


_NC = None


def _get_nc():
    global _NC
    if _NC is None:
        _NC = build_nc()
    return _NC


def kernel(**inputs):
    nc = _get_nc()
    in_maps = make_in_maps(inputs)
    res = bass_utils.run_bass_kernel_spmd(nc, in_maps, core_ids=list(range(NCORES)))
    yf = np.empty((B, C, N), dtype=np.float32)
    for i in range(NCORES):
        b, h = divmod(i, 2)
        yf[b][:, h * NH:(h + 1) * NH] = res.results[i]["y"]
    return yf.reshape(B, C, W, H)
